# revision 63
# baseline (speedup 1.0000x reference)
"""BiLSTM-CRF (Viterbi decode) Trainium2 Bass kernel, 8-core data-parallel.

Full inputs in, full outputs out. Batch (64) is sharded 8 ways; each core runs:
  embedding gather -> input matmuls (gx = x @ Wih^T + b) -> 256-step fused
  fwd+bwd LSTM recurrence -> fc emissions -> Viterbi scan -> batched
  backpointer extraction -> backtrace.

Layout convention on device ("version B"): gate/hidden dims live on SBUF
partitions, batch on the free dim, so ACT/DVE use all 128 lanes.
"""

import os
import sys
import types

for _p in ('/opt/trn_rl_repo', '/root/.axon_site'):
    if _p not in sys.path:
        sys.path.insert(0, _p)

import numpy as np
import ml_dtypes

# ---- NTFF profile hook (lets run_bass_kernel_spmd(trace=True) return timings
# under axon; harmless if already registered or unavailable) ----
def _install_ntff_hook():
    try:
        import antenv
        if 'antenv.axon_hooks' in sys.modules:
            return
        from trn_agent_boot.trn_boot import _ntff_profile_via_ctypes
        m = types.ModuleType('antenv.axon_hooks')
        m._hook = _ntff_profile_via_ctypes('/opt/axon/libaxon_pjrt.so')
        m.get_axon_ntff_profile_hook = lambda: m._hook
        m.set_axon_ntff_profile_hook = lambda h: setattr(m, '_hook', h)
        sys.modules['antenv.axon_hooks'] = m
        antenv.axon_hooks = m
    except Exception:
        pass


_install_ntff_hook()

import concourse.bass as bass
import concourse.tile as tile
from concourse import bacc, mybir
from concourse.bass import IndirectOffsetOnAxis
from concourse.bass_utils import run_bass_kernel_spmd

F32 = mybir.dt.float32
BF16 = mybir.dt.bfloat16
I32 = mybir.dt.int32

# Problem dims (hardcoded per contract)
V, E, HS, T, B = 30000, 256, 512, 256, 64
H = HS // 2          # 256 per-direction hidden
G = 4 * H            # 1024 gate rows per direction
K = 10               # tags
NC_ = 8              # cores
BL = B // NC_        # 8 sequences per core
NBT = BL * T         # 2048 (b,t) columns per core
NSLOT = NBT // 128   # 16 gather slots

# Gate reorder: torch rows [i, f, g, o] -> device order [i, f, o, g]
# (sigmoid block = chunks 0..5, tanh block = chunks 6..7)
_PERM = np.concatenate([
    np.arange(0, 2 * H),          # i, f
    np.arange(3 * H, 4 * H),      # o
    np.arange(2 * H, 3 * H),      # g
])


def _bf(x):
    return np.ascontiguousarray(np.asarray(x, np.float32).astype(ml_dtypes.bfloat16))


def _f32(x):
    return np.ascontiguousarray(np.asarray(x, np.float32))


def _pack_w(wih, whh, bih, bhh):
    """Per direction: returns (w_ih[128, 2*8*128], w_hh[...], bias[128, 8]) in
    lhsT tile layout w[p, kc, mc, m] = W[perm[mc*128+m], kc*128+p]."""
    out = []
    for W in (wih, whh):
        Wp = np.asarray(W, np.float32)[_PERM]          # [G, Kdim]
        Kd = Wp.shape[1]
        t = Wp.reshape(8, 128, Kd // 128, 128)          # [mc, m, kc, p]
        t = np.transpose(t, (3, 2, 0, 1))               # [p, kc, mc, m]
        out.append(t.reshape(128, -1))
    b = (np.asarray(bih, np.float32) + np.asarray(bhh, np.float32))[_PERM]
    b = b.reshape(8, 128).T                             # [p, mc]
    return out[0], out[1], b


def _prep_core(inputs, core):
    """Host-side prep of all per-core device inputs."""
    s = slice(core * BL, (core + 1) * BL)
    inp = np.asarray(inputs['inp'])[s]        # [8, 256] int
    n = np.asarray(inputs['n'])[s].astype(np.int64)

    t_idx = np.arange(T)
    mask = t_idx[None, :] < n[:, None]
    rev = np.where(mask, n[:, None] - 1 - t_idx[None, :], t_idx[None, :])
    tok_rev = np.take_along_axis(inp, rev, axis=1)

    def idx_pack(tok):  # [8,256] -> [128, 16] slot layout (j = s*128+p, j=b*256+t)
        flat = np.asarray(tok, np.int64).reshape(-1)     # j = b*256+t
        return flat.reshape(NSLOT, 128).T.astype(np.int32).copy()

    wf = _pack_w(inputs['W_ih_f'], inputs['W_hh_f'], inputs['b_ih_f'], inputs['b_hh_f'])
    wb = _pack_w(inputs['W_ih_b'], inputs['W_hh_b'], inputs['b_ih_b'], inputs['b_hh_b'])
    w_ih = _bf(np.concatenate([wf[0], wb[0]], axis=1))   # [128, 2*2048]
    w_hh = _bf(np.concatenate([wf[1], wb[1]], axis=1))
    bias = _f32(np.concatenate([wf[2], wb[2]], axis=1))  # [128, 16] (d, mc)

    fcw = np.asarray(inputs['fc_w'], np.float32)         # [10, 512]
    fcw_t = fcw.T.reshape(4, 128, K).transpose(1, 0, 2).reshape(128, 4 * K)
    fcbR = np.tile(np.asarray(inputs['fc_b'], np.float32)[None, :], (128, 1))

    trans = np.asarray(inputs['transition'], np.float32)[:K, :K]  # [prev, cur]
    transR = np.tile(trans.T.reshape(1, K * K), (128, 1)).copy()  # [p, cur*10+prev]

    iotaD = np.tile((9.0 - np.arange(K, dtype=np.float32))[None, :], (128, 1))
    iotaK = np.tile(np.arange(K, dtype=np.float32)[None, :], (128, 1))

    # Blocked Viterbi layout: partition p = l*8 + b. Pass-2 runs 32 steps
    # sp in [0,32) handling scan step u = 16l + sp - 15 (first 16 = warm-up);
    # pass 1 / P6 use s in [0,16) handling u = 16l + s + 1 (== sp - 16).
    LBV = 16
    ll = np.arange(LBV)[:, None, None]                 # [l, 1, 1]
    ss = np.arange(LBV)[None, None, :]                 # [1, 1, s]
    uu = 16 * ll + ss + 1                              # [l, 1, s]
    validm = (uu < n[None, :, None]).astype(np.float32)          # [l, b, s]
    offdiag = -1e4 * (1.0 - np.eye(K, dtype=np.float32))         # [c, q]
    ivbp = (1.0 - validm)[:, :, :, None] * np.arange(K, dtype=np.float32)[None, None, None]
    mask128 = ((uu - 1) < n[None, :, None]).astype(np.float32)   # t = 16l+s
    spp = np.arange(2 * LBV)[None, None, :]            # [1, 1, sp]
    uuW = 16 * ll + spp - 15                           # [l, 1, sp]
    validW = ((uuW >= 1) & (uuW < n[None, :, None])).astype(np.float32)
    ivbaseW = (1.0 - validW)[:, :, :, None, None] * offdiag[None, None, None]
    kk = np.arange(K, dtype=np.float32)
    hypinit = np.tile((np.where(kk[:, None] == kk[None, :], 0.0, -1e4)
                       ).reshape(1, K * K), (128, 1))             # [p, (h,c)]
    hypoh = np.tile((kk[:, None] == kk[None, :]).astype(np.float32)
                    .reshape(1, K * K), (128, 1))                 # [p, (h,k)]

    # hb re-reversal gather rows: out col j=(b,t) <- hb_dram row b*256 + scan_idx
    scan_idx = np.where(mask, n[:, None] - 1 - t_idx[None, :], t_idx[None, :])
    hb_rows = ((scan_idx // 16) * 128 + (scan_idx % 16) * 8
               + np.arange(BL)[:, None]).reshape(-1)
    hb_off = hb_rows.reshape(NSLOT, 128).T.astype(np.int32).copy()

    return {
        'emb': _f32(inputs['emb']),
        'xidx': idx_pack(inp),
        'xridx': idx_pack(tok_rev),
        'w_ih': w_ih, 'w_hh': w_hh, 'bias32': bias,
        'fcw': _bf(fcw_t), 'fcbR': fcbR,
        'ident': np.eye(128, dtype=np.float32),
        'ident_bf': _bf(np.eye(128, dtype=np.float32)),
        'transR': transR, 'iotaD': iotaD, 'iotaK': iotaK,
        'validm': _f32(validm.reshape(128, 16)),
        'validW': _f32(validW.reshape(128, 32)),
        'ivbaseW': _f32(ivbaseW.reshape(128, 32 * K * K)),
        'ivbp': _f32(ivbp.reshape(128, 16 * K)),
        'mask128': _f32(mask128.reshape(128, 16)),
        'hypinit': _f32(hypinit), 'hypoh': _f32(hypoh),
        'hb_off': hb_off,
    }


# ----------------------------------------------------------------------------
# Device kernel
# ----------------------------------------------------------------------------

PHASE = int(os.environ.get('KPHASE', '9'))


def _build():
    nc = bacc.Bacc("TRN2", target_bir_lowering=False, debug=False,
                   num_devices=NC_)

    d_in = {}
    def din(name, shape, dt):
        d_in[name] = nc.dram_tensor(name, list(shape), dt, kind="ExternalInput").ap()
        return d_in[name]

    emb_d = din('emb', [V, E], F32)
    xidx_d = din('xidx', [128, NSLOT], I32)
    xridx_d = din('xridx', [128, NSLOT], I32)
    wih_d = din('w_ih', [128, 2 * 2 * 8 * 128], BF16)
    whh_d = din('w_hh', [128, 2 * 2 * 8 * 128], BF16)
    bias_d = din('bias32', [128, 16], F32)
    fcw_d = din('fcw', [128, 4 * K], BF16)
    fcb_d = din('fcbR', [128, K], F32)
    id_d = din('ident', [128, 128], F32)
    idbf_d = din('ident_bf', [128, 128], BF16)
    trans_d = din('transR', [128, K * K], F32)
    iotaD_d = din('iotaD', [128, K], F32)
    iotaK_d = din('iotaK', [128, K], F32)
    validm_d = din('validm', [128, 16], F32)
    validW_d = din('validW', [128, 32], F32)
    ivbaseW_d = din('ivbaseW', [128, 32 * K * K], F32)
    ivbp_d = din('ivbp', [128, 16 * K], F32)
    mask128_d = din('mask128', [128, 16], F32)
    hypinit_d = din('hypinit', [128, K * K], F32)
    hypoh_d = din('hypoh', [128, K * K], F32)
    hboff_d = din('hb_off', [128, NSLOT], I32)

    out_d = nc.dram_tensor('out', [BL, T], F32, kind="ExternalOutput").ap()
    dbg_d = nc.dram_tensor('dbg', [128, 64], F32, kind="ExternalOutput").ap()

    SIG = mybir.ActivationFunctionType.Sigmoid
    TANH = mybir.ActivationFunctionType.Tanh
    AL = mybir.AluOpType
    AX = mybir.AxisListType

    with tile.TileContext(nc) as tc:
        from contextlib import ExitStack
        ctx = ExitStack()
        cpool = ctx.enter_context(tc.tile_pool(name="consts", bufs=1))
        state = ctx.enter_context(tc.tile_pool(name="state", bufs=1))
        gather_p = ctx.enter_context(tc.tile_pool(name="gather", bufs=2))
        scratch = ctx.enter_context(tc.tile_pool(name="scratch", bufs=2))
        vit_p = ctx.enter_context(tc.tile_pool(name="vit", bufs=2))
        vbig = ctx.enter_context(tc.tile_pool(name="vbig", bufs=1))
        ps_tr = ctx.enter_context(tc.tile_pool(name="ps_tr", bufs=1, space="PSUM"))
        ps_mm = ctx.enter_context(tc.tile_pool(name="ps_mm", bufs=1, space="PSUM"))
        ps_g = ctx.enter_context(tc.tile_pool(name="ps_g", bufs=2, space="PSUM"))
        ps_gb = ctx.enter_context(tc.tile_pool(name="ps_gb", bufs=2, space="PSUM"))
        ps_fc = ctx.enter_context(tc.tile_pool(name="ps_fc", bufs=2, space="PSUM"))
        dram_p = ctx.enter_context(tc.tile_pool(name="dram", bufs=1, space="DRAM"))

        hb_dram_t = dram_p.tile([NBT, H], BF16)
        feats_dram_t = dram_p.tile([BL * T * K], F32)
        vb_dram_t = dram_p.tile([24576], F32)   # scratch for SBUF bounces
        hb_dram = hb_dram_t[:]
        feats_dram = feats_dram_t[:]
        vb_dram = vb_dram_t[:]

        def load_const(dram, shape, dt, tag):
            t = cpool.tile(shape, dt, tag=tag)
            nc.sync.dma_start(t[:], dram)
            return t

        wih = load_const(wih_d[:], [128, 4096], BF16, tag='wih')
        whh = load_const(whh_d[:], [128, 4096], BF16, tag='whh')
        bias = load_const(bias_d[:], [128, 16], F32, tag='bias')
        fcw = load_const(fcw_d[:], [128, 4 * K], BF16, tag='fcw')
        fcbR = load_const(fcb_d[:], [128, K], F32, tag='fcbR')
        ident = load_const(id_d[:], [128, 128], F32, tag='ident')
        ident_bf = load_const(idbf_d[:], [128, 128], BF16, tag='ident_bf')
        transR = load_const(trans_d[:], [128, K * K], F32, tag='transR')
        iotaD = load_const(iotaD_d[:], [128, K], F32, tag='iotaD')
        iotaK = load_const(iotaK_d[:], [128, K], F32, tag='iotaK')
        validm = load_const(validm_d[:], [128, 16], F32, tag='validm')
        validW = load_const(validW_d[:], [128, 32], F32, tag='validW')
        ivbaseW = load_const(ivbaseW_d[:], [128, 32 * K * K], F32, tag='ivbaseW')
        ivbp = load_const(ivbp_d[:], [128, 16 * K], F32, tag='ivbp')
        mask128 = load_const(mask128_d[:], [128, 16], F32, tag='mask128')
        hypinit = load_const(hypinit_d[:], [128, K * K], F32, tag='hypinit')
        hypoh = load_const(hypoh_d[:], [128, K * K], F32, tag='hypoh')
        xidx = load_const(xidx_d[:], [128, NSLOT], I32, tag='xidx')
        xridx = load_const(xridx_d[:], [128, NSLOT], I32, tag='xridx')
        hboff = load_const(hboff_d[:], [128, NSLOT], I32, tag='hboff')

        wih_r = wih[:].rearrange("p (d kc mc m) -> p d kc mc m", d=2, kc=2, mc=8)
        whh_r = whh[:].rearrange("p (d kc mc m) -> p d kc mc m", d=2, kc=2, mc=8)
        fcw_r = fcw[:].rearrange("p (c k) -> p c k", c=4)

        # ---- P1: embedding gather + transpose to x^T (E on partitions) ----
        gx_ctx = ExitStack()
        gxpool = gx_ctx.enter_context(tc.tile_pool(name="gxp", bufs=1))
        x_ctx = ExitStack()
        xpool = x_ctx.enter_context(tc.tile_pool(name="xp", bufs=1))
        x_bf = xpool.tile([128, 2 * 2 * NBT], BF16)   # [p, dir, ec, bt]
        xbf_r = x_bf[:].rearrange("p (d e n) -> p d e n", d=2, e=2)
        for d, idxt in ((0, xidx), (1, xridx)):
            for s_ in range(NSLOT):
                xs = gather_p.tile([128, E], F32, tag="xslot")
                nc.gpsimd.indirect_dma_start(
                    out=xs[:], out_offset=None, in_=emb_d,
                    in_offset=IndirectOffsetOnAxis(ap=idxt[:, s_:s_ + 1], axis=0),
                )
                for ec in range(2):
                    pt = ps_tr.tile([128, 128], F32, tag="ptr")
                    nc.tensor.transpose(out=pt[:], in_=xs[:, ec * 128:(ec + 1) * 128],
                                        identity=ident[:])
                    nc.vector.tensor_copy(
                        out=xbf_r[:, d, ec, s_ * 128:(s_ + 1) * 128], in_=pt[:])

        # ---- P2: gx = x @ Wih^T + bias (both dirs), bf16 store ----
        gx = gxpool.tile([128, 2 * 8 * NBT], BF16)     # [p, dir, mc, bt]
        gx_r = gx[:].rearrange("p (d mc n) -> p d mc n", d=2, mc=8)
        gx_rt = gx[:].rearrange("p (d mc b t) -> p d mc b t", d=2, mc=8, b=BL)
        NB = NBT // 512
        for d in range(2):
            for mc in range(8):
                for nb in range(NB):
                    pm = ps_mm.tile([128, 512], F32, tag="pmm")
                    for kc in range(2):
                        nc.tensor.matmul(
                            out=pm[:], lhsT=wih_r[:, d, kc, mc, :],
                            rhs=xbf_r[:, d, kc, nb * 512:(nb + 1) * 512],
                            start=(kc == 0), stop=(kc == 1))
                    nc.vector.tensor_scalar(
                        out=gx_r[:, d, mc, nb * 512:(nb + 1) * 512], in0=pm[:],
                        scalar1=bias[:, d * 8 + mc:d * 8 + mc + 1], scalar2=None,
                        op0=AL.add)

        x_ctx.close()

        # ---- P3: fused fwd+bwd LSTM scan ----
        # Per-step critical chain: MM(accumulate onto PSUM-preloaded gx) ->
        # sig(ifo) -> fused i*g|f*c mul -> c add -> tanh(c) -> h mul.
        # tanh(g) runs on ACT during the ifo matmuls; gx preload copies run
        # on DVE off the chain (double-buffered PSUM).
        hall = state.tile([128, 2 * 2 * (T + 1) * BL], BF16)  # [p, d, kc, t, b]
        hall_r = hall[:].rearrange("p (d kc t b) -> p d kc t b", d=2, kc=2, t=T + 1)
        # tgc: chunks [d][0:2] = tanh(g), chunks [d][2:4] = c state
        tgc = state.tile([128, 2 * 4 * BL], F32)
        tgc_r = tgc[:].rearrange("p (d c b) -> p d c b", d=2, c=4)
        nc.vector.memset(hall_r[:, :, :, 0, :], 0.0)
        nc.vector.memset(tgc[:], 0.0)

        for t in range(T):
            # g-gate matmuls first: their add + tanh overlap the i,f,o matmuls
            pga = ps_g.tile([128, 2 * 2 * BL], F32, tag="pga")   # g gates
            pgb = ps_gb.tile([128, 2 * 6 * BL], F32, tag="pgb")  # i,f,o gates
            pga_r = pga[:].rearrange("p (d c b) -> p d c b", d=2, c=2)
            pgb_r = pgb[:].rearrange("p (d c b) -> p d c b", d=2, c=6)
            for d in range(2):
                for mc in (6, 7):
                    for kc in range(2):
                        nc.tensor.matmul(
                            out=pga_r[:, d, mc - 6, :], lhsT=whh_r[:, d, kc, mc, :],
                            rhs=hall_r[:, d, kc, t, :],
                            start=(kc == 0), stop=(kc == 1))
            gg = scratch.tile([128, 2 * 2 * BL], F32, tag="gg")
            gg_r = gg[:].rearrange("p (d c b) -> p d c b", d=2, c=2)
            nc.vector.tensor_tensor(out=gg_r[:, :, :, :], in0=pga_r[:, :, :, :],
                                    in1=gx_rt[:, :, 6:8, :, t], op=AL.add)
            nc.scalar.activation(out=tgc_r[:, :, 0:2, :], in_=gg_r[:, :, :, :],
                                 func=TANH)
            for d in range(2):
                for mc in range(6):
                    for kc in range(2):
                        nc.tensor.matmul(
                            out=pgb_r[:, d, mc, :], lhsT=whh_r[:, d, kc, mc, :],
                            rhs=hall_r[:, d, kc, t, :],
                            start=(kc == 0), stop=(kc == 1))
            gsb = scratch.tile([128, 2 * 6 * BL], F32, tag="gsb")
            gsb_r = gsb[:].rearrange("p (d c b) -> p d c b", d=2, c=6)
            nc.vector.tensor_tensor(out=gsb_r[:, :, :, :], in0=pgb_r[:, :, :, :],
                                    in1=gx_rt[:, :, 0:6, :, t], op=AL.add)
            sig = scratch.tile([128, 2 * 6 * BL], F32, tag="sig")
            sig_r = sig[:].rearrange("p (d c b) -> p d c b", d=2, c=6)
            nc.scalar.activation(out=sig_r[:, :, :, :], in_=gsb_r[:, :, :, :], func=SIG)

            # prod[d][0:2] = i*tanh(g), prod[d][2:4] = f*c
            prod = scratch.tile([128, 2 * 4 * BL], F32, tag="prod")
            prod_r = prod[:].rearrange("p (d c b) -> p d c b", d=2, c=4)
            nc.vector.tensor_mul(out=prod_r[:, :, :, :], in0=sig_r[:, :, 0:4, :],
                                 in1=tgc_r[:, :, :, :])
            nc.vector.tensor_add(out=tgc_r[:, :, 2:4, :], in0=prod_r[:, :, 0:2, :],
                                 in1=prod_r[:, :, 2:4, :])
            tc_ = scratch.tile([128, 2 * 2 * BL], F32, tag="tc")
            tc_r = tc_[:].rearrange("p (d c b) -> p d c b", d=2, c=2)
            nc.scalar.activation(out=tc_r[:, :, :, :], in_=tgc_r[:, :, 2:4, :], func=TANH)
            nc.vector.tensor_mul(out=hall_r[:, :, :, t + 1, :], in0=sig_r[:, :, 4:6, :],
                                 in1=tc_r[:, :, :, :])

        gx_ctx.close()

        # ---- P4: hb re-reversal (DRAM bounce + indirect gather + transpose),
        #          then fc emissions ----
        # transpose hb (hid-on-partitions) -> scan-row tiles [bt, hid], dump to DRAM
        for s_ in range(NSLOT):
            hbs = gather_p.tile([128, H], BF16, tag="hbs")
            for ec in range(2):
                pt = ps_tr.tile([128, 128], BF16, tag="ptr")
                nc.tensor.transpose(
                    out=pt[:],
                    in_=hall_r[:, 1, ec, 1 + s_ * 16:1 + (s_ + 1) * 16, :],
                    identity=ident_bf[:])
                nc.vector.tensor_copy(out=hbs[:, ec * 128:(ec + 1) * 128], in_=pt[:])
            nc.sync.dma_start(out=hb_dram[s_ * 128:(s_ + 1) * 128, :], in_=hbs[:])
        hbT = state.tile([128, 2 * NBT], BF16)   # [p(hid), kc, bt]
        hbT_r = hbT[:].rearrange("p (kc n) -> p kc n", kc=2)
        for s_ in range(NSLOT):
            hs = gather_p.tile([128, H], BF16, tag="hslot")
            nc.gpsimd.indirect_dma_start(
                out=hs[:], out_offset=None, in_=hb_dram,
                in_offset=IndirectOffsetOnAxis(ap=hboff[:, s_:s_ + 1], axis=0))
            for ec in range(2):
                pt = ps_tr.tile([128, 128], BF16, tag="ptr")
                nc.tensor.transpose(out=pt[:], in_=hs[:, ec * 128:(ec + 1) * 128],
                                    identity=ident_bf[:])
                nc.vector.tensor_copy(out=hbT_r[:, ec, s_ * 128:(s_ + 1) * 128], in_=pt[:])

        feats_sb = state.tile([128, 16 * K], F32)   # [p, mt, k], bt = mt*128+p
        feats_r = feats_sb[:].rearrange("p (m k) -> p m k", m=16)
        for mt in range(16):
            b_, th = mt // 2, mt % 2
            pf = ps_fc.tile([128, K], F32, tag="pfc")
            for c4 in range(4):
                if c4 < 2:
                    lhs = hall_r[:, 0, c4, 1 + th * 128:1 + (th + 1) * 128, b_]
                else:
                    lhs = hbT_r[:, c4 - 2, mt * 128:(mt + 1) * 128]
                nc.tensor.matmul(out=pf[:], lhsT=lhs, rhs=fcw_r[:, c4, :],
                                 start=(c4 == 0), stop=(c4 == 3))
            nc.vector.tensor_tensor(out=feats_r[:, mt, :], in0=pf[:],
                                    in1=fcbR[:, :], op=AL.add)

        # relayout feats -> DRAM flat [b, t, k]
        nc.sync.dma_start(
            out=feats_dram[0:BL * T * K]
                .rearrange("(b th p k) -> p b th k", b=BL, th=2, p=128),
            in_=feats_r[:, :, :].rearrange("p (b th) k -> p b th k", b=BL))

        # ---- P5: blocked Viterbi forward on 128 partitions ----
        # partition p = l*8 + b; step s of block l handles scan step
        # u = 16l + s + 1 (u=256 is an identity pad step).
        LB = 16

        # feats128W[p=(l,b), (sp,c)] = feats[b, 16l+sp-15, c], sp in [0,32).
        # Staged through DRAM in (l b sp c) order so the SBUF write is one
        # full-partition DMA (partition-offset DMA writes race in tracking).
        SW = 2 * LB
        feats128W = state.tile([128, SW * K], F32)
        fW_r = feats128W[:].rearrange("p (s c) -> p s c", s=SW)
        fd_v = feats_dram[0:BL * T * K].rearrange("(b u c) -> b u c", b=BL, u=T)
        nc.vector.memset(feats128W[:], 0.0)
        for l in range(LB):
            sp0 = 15 if l == 0 else 0          # u >= 0
            sp1 = 31 if l == LB - 1 else SW    # u <= 255
            u0 = 16 * l + sp0 - 15
            nc.sync.dma_start(
                out=feats128W[l * BL:(l + 1) * BL, sp0 * K:sp1 * K]
                    .rearrange("p (s c) -> p s c", s=sp1 - sp0),
                in_=fd_v[:, u0:u0 + sp1 - sp0, :])

        # base128W[p, sp, c, q] = valid*(trans[q->c] + ob[c]) + (1-valid)*ID
        base128W = state.tile([128, SW * K * K], F32)
        b_rW = base128W[:].rearrange("p (s c q) -> p s c q", s=SW, c=K)
        trans_cq = transR[:, :].rearrange("p (c q) -> p c q", c=K)
        nc.vector.tensor_tensor(
            out=b_rW[:, :, :, :],
            in0=fW_r[:, :, :].unsqueeze(3).broadcast_to((128, SW, K, K)),
            in1=trans_cq.unsqueeze(1).broadcast_to((128, SW, K, K)), op=AL.add)
        nc.vector.tensor_tensor(
            out=b_rW[:, :, :, :], in0=b_rW[:, :, :, :],
            in1=validW[:, :].unsqueeze(2).unsqueeze(3)
                .broadcast_to((128, SW, K, K)),
            op=AL.mult)
        nc.vector.tensor_tensor(
            out=b_rW[:, :, :, :], in0=b_rW[:, :, :, :],
            in1=ivbaseW[:, :].rearrange("p (s c q) -> p s c q", s=SW, c=K),
            op=AL.add)
        # pass-1 / P6 view: b_r[:, s, ...] = step u = 16l+s+1  (sp = s+16)
        b_r = base128W[:, LB * K * K:].rearrange("p (s c q) -> p s c q", s=LB, c=K)

        # pass 1: per-block max-plus matrices via 10 entry hypotheses
        src = hypinit[:].rearrange("p (h q) -> p h q", h=K)
        for s in range(LB):
            sh = vit_p.tile([128, K * K * K], F32, tag="sh")
            sh_r = sh[:].rearrange("p (h c q) -> p h c q", h=K, c=K)
            nc.vector.tensor_tensor(
                out=sh_r[:, :, :, :],
                in0=src.unsqueeze(2).broadcast_to((128, K, K, K)),
                in1=b_r[:, s, :, :].unsqueeze(1).broadcast_to((128, K, K, K)),
                op=AL.add)
            pr = vit_p.tile([128, K * K], F32, tag="preh")
            pr_r = pr[:].rearrange("p (h c) -> p h c", h=K)
            nc.vector.tensor_reduce(out=pr_r[:, :, :], in_=sh_r[:, :, :, :],
                                    axis=AX.X, op=AL.max)
            src = pr_r
        # M^T layout [p, c, h] for the stitch reduce
        Mt = state.tile([128, K * K], F32)
        Mt_r = Mt[:].rearrange("p (c h) -> p c h", c=K)
        nc.vector.tensor_copy(out=Mt_r[:, :, :],
                              in_=pr[:].rearrange("p (h c) -> p c h", h=K))

        # stitch: sequential entry vectors, on partitions 0:8 with the block
        # matrices shuttled to the free dim (engine APs must start at p=0)
        MtF = state.tile([128, LB * K * K], F32)   # [b, (l, c, h)], rows 0:8
        MtF_r = MtF[:].rearrange("p (l c h) -> p l c h", l=LB, c=K)
        dA = vb_dram[0:128 * K * K]
        nc.sync.dma_start(out=dA.rearrange("(p ch) -> p ch", ch=K * K), in_=Mt[:])
        nc.sync.dma_start(
            out=MtF[0:BL, :].rearrange("p (l ch) -> p l ch", l=LB),
            in_=dA.rearrange("(l b ch) -> b l ch", l=LB, b=BL))
        entF = state.tile([128, LB * K], F32)      # [b, (l, c)], rows 0:8
        nc.sync.dma_start(
            out=entF[0:BL, 0:K],
            in_=feats_dram[0:BL * T * K]
                .rearrange("(b t c) -> b t c", b=BL, t=T)[:, 0, :])
        for l in range(LB - 1):
            se = vit_p.tile([128, K * K], F32, tag="se")
            se_r = se[:].rearrange("p (c h) -> p c h", c=K)
            nc.vector.tensor_tensor(
                out=se_r[0:BL, :, :],
                in0=entF[0:BL, l * K:(l + 1) * K].unsqueeze(1)
                    .broadcast_to((BL, K, K)),
                in1=MtF_r[0:BL, l, :, :], op=AL.add)
            nc.vector.tensor_reduce(
                out=entF[0:BL, (l + 1) * K:(l + 2) * K],
                in_=se_r[0:BL, :, :], axis=AX.X, op=AL.max)
        # warm-up entries: block l starts from ent_{l-1} at u = 16(l-1);
        # block 0 starts from alpha0 (its warm-up steps are all identity)
        # pre2init[(l,b)] = ent_{max(l-1,0)}  (ent_0 == alpha0)
        pre2init = state.tile([128, K], F32)
        dB = vb_dram[12800:12800 + 128 * K]
        nc.sync.dma_start(
            out=dB[0:BL * K].rearrange("(b c) -> b c", c=K),
            in_=entF[0:BL, 0:K])
        nc.sync.dma_start(
            out=dB[BL * K:].rearrange("(l b c) -> b l c", l=LB - 1, b=BL),
            in_=entF[0:BL, 0:(LB - 1) * K].rearrange("p (l c) -> p l c", l=LB - 1))
        nc.sync.dma_start(
            out=pre2init[:, :],
            in_=dB.rearrange("(lb c) -> lb c", c=K))

        # pass 2: 32 steps (16 warm-up + 16 live), storing pre_{16l+j} for
        # j in [0,17) in preall32 slot j
        preall32 = state.tile([128, 17 * K], F32)
        pa_r = preall32[:].rearrange("p (s c) -> p s c", s=17)
        prev2d = pre2init[:, :]
        for sp in range(SW):
            s2 = vit_p.tile([128, K * K], F32, tag="s2")
            s2_r = s2[:].rearrange("p (c q) -> p c q", c=K)
            nc.vector.tensor_tensor(
                out=s2_r[:, :, :],
                in0=prev2d.unsqueeze(1).broadcast_to((128, K, K)),
                in1=b_rW[:, sp, :, :], op=AL.add)
            if sp >= 15:
                nc.vector.tensor_reduce(out=pa_r[:, sp - 15, :], in_=s2_r[:, :, :],
                                        axis=AX.X, op=AL.max)
                prev2d = pa_r[:, sp - 15, :]
            else:
                pw = vit_p.tile([128, K], F32, tag="pw")
                nc.vector.tensor_reduce(out=pw[:, :], in_=s2_r[:, :, :],
                                        axis=AX.X, op=AL.max)
                prev2d = pw[:, :]

        # ---- P6: batched backpointer extraction in the (l,b) layout ----
        # preall32 slot s = pre_{16l+s} = pre_{u-1} for u = 16l+s+1
        pp_r = pa_r[:, 0:LB, :]
        sX = vbig.tile([128, LB * K * K], F32, tag="sX")
        sX_r = sX[:].rearrange("p (s c q) -> p s c q", s=LB, c=K)
        nc.vector.tensor_tensor(
            out=sX_r[:, :, :, :],
            in0=pp_r[:, :, :].unsqueeze(2).broadcast_to((128, LB, K, K)),
            in1=trans_cq.unsqueeze(1).broadcast_to((128, LB, K, K)), op=AL.add)
        mX = vbig.tile([128, LB * K], F32, tag="mX")
        mX_r = mX[:].rearrange("p (s c) -> p s c", s=LB)
        nc.vector.tensor_reduce(out=mX_r[:, :, :], in_=sX_r[:, :, :, :],
                                axis=AX.X, op=AL.max)
        eq = vbig.tile([128, LB * K * K], F32, tag="eq")
        eq_r = eq[:].rearrange("p (s c q) -> p s c q", s=LB, c=K)
        nc.vector.tensor_tensor(
            out=eq_r[:, :, :, :], in0=sX_r[:, :, :, :],
            in1=mX_r[:, :, :].unsqueeze(3).broadcast_to((128, LB, K, K)),
            op=AL.is_equal)
        nc.vector.tensor_tensor(
            out=eq_r[:, :, :, :], in0=eq_r[:, :, :, :],
            in1=iotaD[:, :].unsqueeze(1).unsqueeze(1)
                .broadcast_to((128, LB, K, K)),
            op=AL.mult)
        bp128 = state.tile([128, LB * K], F32)
        bp_v = bp128[:].rearrange("p (s c) -> p s c", s=LB)
        nc.vector.tensor_reduce(out=bp_v[:, :, :], in_=eq_r[:, :, :, :],
                                axis=AX.X, op=AL.max)
        nc.vector.tensor_scalar(out=bp128[:], in0=bp128[:], scalar1=-1.0,
                                scalar2=9.0, op0=AL.mult, op1=AL.add)
        nc.vector.tensor_tensor(
            out=bp_v[:, :, :], in0=bp_v[:, :, :],
            in1=validm[:, :].unsqueeze(2).broadcast_to((128, LB, K)), op=AL.mult)
        nc.vector.tensor_tensor(
            out=bp_v[:, :, :], in0=bp_v[:, :, :],
            in1=ivbp[:, :].rearrange("p (s c) -> p s c", s=LB), op=AL.add)

        # ---- P7: end-tag, blocked backtrace, stitch, extract ----
        # end tag from block 15, s=15 (pre_256 == pre_255 via identity pad);
        # computed on partitions 0:8 via a bounce DMA
        pend = state.tile([128, K], F32)
        dE = vb_dram[16000:16000 + 128 * K]
        nc.sync.dma_start(out=dE.rearrange("(lb c) -> lb c", c=K),
                          in_=preall32[:, 16 * K:17 * K])
        nc.sync.dma_start(out=pend[0:BL, :],
                          in_=dE[120 * K:].rearrange("(b c) -> b c", c=K))
        peF = state.tile([128, LB * K], F32)   # [b, (l, h)] entry onehots
        mvE = vit_p.tile([128, 1], F32, tag="mvE")
        nc.vector.tensor_reduce(out=mvE[0:BL, :], in_=pend[0:BL, :],
                                axis=AX.X, op=AL.max)
        eqE = vit_p.tile([128, K], F32, tag="eqE")
        nc.vector.tensor_tensor(out=eqE[0:BL, :], in0=pend[0:BL, :],
                                in1=mvE[0:BL, :].broadcast_to((BL, K)),
                                op=AL.is_equal)
        nc.vector.tensor_mul(out=eqE[0:BL, :], in0=eqE[0:BL, :],
                             in1=iotaD[0:BL, :])
        eT = vit_p.tile([128, 1], F32, tag="eT")
        nc.vector.tensor_reduce(out=eT[0:BL, :], in_=eqE[0:BL, :],
                                axis=AX.X, op=AL.max)
        nc.vector.tensor_scalar(out=eT[0:BL, :], in0=eT[0:BL, :],
                                scalar1=-1.0, scalar2=9.0, op0=AL.mult, op1=AL.add)
        nc.vector.tensor_tensor(out=peF[0:BL, (LB - 1) * K:LB * K],
                                in0=iotaK[0:BL, :],
                                in1=eT[0:BL, :].broadcast_to((BL, K)),
                                op=AL.is_equal)

        # blocked backtrace under 10 exit hypotheses, all blocks at once
        tags = state.tile([128, LB * K], F32)   # [p, (s, h)] = tag_{16l+s}|hyp h
        tg_r = tags[:].rearrange("p (s h) -> p s h", s=LB)
        oh_cur = hypoh[:].rearrange("p (h k) -> p h k", h=K)
        for s in range(LB - 1, -1, -1):
            sel = vit_p.tile([128, K * K], F32, tag="sel")
            sel_r = sel[:].rearrange("p (h k) -> p h k", h=K)
            nc.vector.tensor_tensor(
                out=sel_r[:, :, :], in0=oh_cur,
                in1=bp_v[:, s, :].unsqueeze(1).broadcast_to((128, K, K)),
                op=AL.mult)
            nc.vector.tensor_reduce(out=tg_r[:, s, :], in_=sel_r[:, :, :],
                                    axis=AX.X, op=AL.max)
            if s > 0:
                ohn = vit_p.tile([128, K * K], F32, tag="ohn")
                ohn_r = ohn[:].rearrange("p (h k) -> p h k", h=K)
                nc.vector.tensor_tensor(
                    out=ohn_r[:, :, :],
                    in0=iotaK[:, :].unsqueeze(1).broadcast_to((128, K, K)),
                    in1=tg_r[:, s, :].unsqueeze(2).broadcast_to((128, K, K)),
                    op=AL.is_equal)
                oh_cur = ohn_r

        # stitch true entry tags backwards across blocks (on partitions 0:8)
        tagsS0 = state.tile([128, LB * K], F32)   # [b, (l, h)] = tags at s=0
        dC = vb_dram[17000:17000 + 128 * K]
        nc.sync.dma_start(out=dC.rearrange("(lb c) -> lb c", c=K),
                          in_=tags[:, 0:K])
        nc.sync.dma_start(
            out=tagsS0[0:BL, :].rearrange("p (l c) -> p l c", l=LB),
            in_=dC.rearrange("(l b c) -> b l c", l=LB, b=BL))
        for l in range(LB - 2, -1, -1):
            sel2 = vit_p.tile([128, K], F32, tag="sel2")
            nc.vector.tensor_mul(out=sel2[0:BL, :],
                                 in0=tagsS0[0:BL, (l + 1) * K:(l + 2) * K],
                                 in1=peF[0:BL, (l + 1) * K:(l + 2) * K])
            tt_ = vit_p.tile([128, 1], F32, tag="tt")
            nc.vector.tensor_reduce(out=tt_[0:BL, :], in_=sel2[0:BL, :],
                                    axis=AX.X, op=AL.max)
            nc.vector.tensor_tensor(
                out=peF[0:BL, l * K:(l + 1) * K], in0=iotaK[0:BL, :],
                in1=tt_[0:BL, :].broadcast_to((BL, K)), op=AL.is_equal)
        pe128 = state.tile([128, K], F32)
        dD = vb_dram[18500:18500 + 128 * K]
        nc.sync.dma_start(
            out=dD.rearrange("(l b c) -> b l c", l=LB, b=BL),
            in_=peF[0:BL, :].rearrange("p (l c) -> p l c", l=LB))
        nc.sync.dma_start(out=pe128[:, :],
                          in_=dD.rearrange("(lb c) -> lb c", c=K))

        # final extraction: select each block's true-hypothesis row, mask, out
        selF = vbig.tile([128, LB * K], F32, tag="selF")
        sf_r = selF[:].rearrange("p (s h) -> p s h", s=LB)
        nc.vector.tensor_tensor(
            out=sf_r[:, :, :], in0=tg_r[:, :, :],
            in1=pe128[:, :].unsqueeze(1).broadcast_to((128, LB, K)), op=AL.mult)
        bestv = state.tile([128, LB], F32)
        nc.vector.tensor_reduce(out=bestv[:, :], in_=sf_r[:, :, :],
                                axis=AX.X, op=AL.max)
        nc.vector.tensor_mul(out=bestv[:, :], in0=bestv[:, :], in1=mask128[:, :])
        for l in range(LB):
            nc.sync.dma_start(out=out_d[:, 16 * l:16 * (l + 1)],
                              in_=bestv[8 * l:8 * (l + 1), :])

        # debug dump: [0:10]=preall32 slot16, [10:20]=pend, [20:21]=eT,
        # [21:31]=pe128, [31:41]=tagsS0 row-slice, [41:51]=peF row-slice
        dbg = state.tile([128, 64], F32)
        nc.vector.memset(dbg[:], 0.0)
        nc.vector.tensor_copy(out=dbg[:, 0:10], in_=preall32[:, 160:170])
        nc.vector.tensor_copy(out=dbg[:, 10:20], in_=pend[:, :])
        nc.vector.tensor_copy(out=dbg[:, 20:21], in_=eT[:, :])
        nc.vector.tensor_copy(out=dbg[:, 21:31], in_=pe128[:, :])
        nc.vector.tensor_copy(out=dbg[0:BL, 31:41], in_=tagsS0[0:BL, 3 * K:4 * K])
        nc.vector.tensor_copy(out=dbg[0:BL, 41:51], in_=peF[0:BL, 3 * K:4 * K])
        nc.sync.dma_start(out=dbg_d, in_=dbg[:])
        ctx.close()

    nc.compile()
    return nc


_NC_CACHE = None


def _get_nc():
    global _NC_CACHE
    if _NC_CACHE is None:
        _NC_CACHE = _build()
    return _NC_CACHE


TRACE = False
LAST_EXEC_NS = None


def kernel(**inputs) -> np.ndarray:
    global LAST_EXEC_NS
    nc = _get_nc()
    in_maps = [_prep_core(inputs, c) for c in range(NC_)]
    res = run_bass_kernel_spmd(nc, in_maps, list(range(NC_)), trace=TRACE)
    LAST_EXEC_NS = res.exec_time_ns
    out = np.concatenate([res.results[c]['out'] for c in range(NC_)], axis=0)
    return out.astype(np.float32)


if __name__ == '__main__':
    _build()
    print("build ok")

